# revision 1
# baseline (speedup 1.0000x reference)
"""Contrastive loss on Trainium2 (8 NeuronCores, SPMD, Bass/Tile).

Math
----
reference:
    norms[i,j] = ||x_i||^2 + ||x_j||^2 - 2 x_i.x_j
    pos = sum((eq - I) * norms) / cnt_pos          eq[i,j] = [y_i == y_j]
    neg = sum((1 - eq) * relu(1 - norms)) / cnt_neg
    loss = (pos + neg) / 2

Device trick: for each PSUM tile of the pair matrix we accumulate, via two
matmuls into the same PSUM region,

    u[i,j] = norms[i,j] - 1 + BIG * eq[i,j]          (BIG = 4096 >> max norms)

  - matmul 1 (K=128): lhsT = -2 x_i^T, rhs = x_j^T   -> -2 G
  - matmul 2 (K=45):  lhsT = [onehot; 1; sq_i - 1], rhs = [BIG*onehot; sq_j; 1]
                      -> BIG*eq + sq_j + (sq_i - 1)

Both masked sums then come out of u with ONE fused instruction each:
    pos:  sum relu(u + (1-BIG))  = sum_{eq=1} norms        (ACT, accum_out)
    neg:  sum min(u, 0)          = -sum_{eq=0} relu(1-norms) (DVE, accum_out)
    neg (ACT variant): sum relu(-u) = +sum_{eq=0} relu(1-norms)

Work halving (symmetry): with 128-row blocks r and 128-col blocks c (64 of
each), let d = (c - r) mod 64. The matrix is symmetric, so summing blocks
d=0 (weight 1), d=1..31 (weight 2), d=32 (weight 1; both mirror copies are
visited) covers every ordered pair exactly once. Each row-block therefore
processes a contiguous circular span of 33*128 = 4224 columns.

Sharding: core k owns global rows [1024k, 1024(k+1)). Its 8 row-blocks need
the circular column window [1024k, 1024k + 5120) — the host ships that
window per-core ("rolled" columns), so the device program is identical on
every core (pure SPMD). Per-core outputs are per-partition partial sums;
the host applies block weights / counts and reduces (O(N) work).
"""

import numpy as np
from contextlib import ExitStack

import concourse.bass as bass
import concourse.bacc as bacc
import concourse.tile as tile
from concourse import mybir
from concourse.bass_utils import run_bass_kernel_spmd

N, D, C = 8192, 128, 43
MARGIN = 1.0
BIG = 4096.0
P = 128
NCORES = 8
ROWS_PER_CORE = N // NCORES           # 1024
RB = ROWS_PER_CORE // P               # 8 row-blocks per core
LOCAL_COLS = ROWS_PER_CORE + 32 * P   # 5120: own rows + 32 blocks ahead
AUGK = C + 4                          # 47: onehot + 2x(sq hi/lo) rows

# Per row-block jj (local col base b = 128*jj):
#   d0    : [b, b+128)            weight 1  (packed into small tiles)
#   chunkA: [b+128, b+2176)       FD 2048, weight 2
#   chunkB: [b+2176, b+4096)      FD 1920, weight 2
#   d32   : [b+4096, b+4224)      weight 1  (packed into small tiles)
NPART = 2 * RB + RB // 2              # 16 main units + 4 small tiles = 20
UNIT_W = [2.0] * (2 * RB) + [1.0] * (RB // 2)
# units whose NEG pass runs on ACT (as +relu(-u)) instead of DVE (as min(u,0)).
# ACT gets the even mains (FD 2048) + 2 smalls; DVE the odd mains + 2 smalls.
NEG_ON_ACT = frozenset({0, 2, 4, 6, 8, 10, 12, 14, 16, 17})

_cache = {}
TRACE = False


def _build_bass():
    f32 = mybir.dt.float32
    bf16 = mybir.dt.bfloat16
    nc = bacc.Bacc("TRN2", target_bir_lowering=False, debug=False)

    rhs_x = nc.dram_tensor("rhs_x", [P, LOCAL_COLS], bf16, kind="ExternalInput").ap()
    aug_r = nc.dram_tensor("aug_r", [AUGK, LOCAL_COLS], bf16, kind="ExternalInput").ap()
    lhs_m2 = nc.dram_tensor("lhs_m2", [P, ROWS_PER_CORE], bf16, kind="ExternalInput").ap()
    aug_l = nc.dram_tensor("aug_l", [AUGK, ROWS_PER_CORE], bf16, kind="ExternalInput").ap()
    neg_out = nc.dram_tensor("neg_out", [P, NPART], f32, kind="ExternalOutput").ap()

    relu = mybir.ActivationFunctionType.Relu
    alu_min = mybir.AluOpType.min
    alu_add = mybir.AluOpType.add

    with tile.TileContext(nc) as tc:
        with ExitStack() as ctx:
            const = ctx.enter_context(tc.tile_pool(name="const", bufs=1))
            psum = ctx.enter_context(tc.tile_pool(name="psum", bufs=2, space="PSUM"))
            scr_a = ctx.enter_context(tc.tile_pool(name="scr_a", bufs=2))
            scr_v = ctx.enter_context(tc.tile_pool(name="scr_v", bufs=2))

            xt = const.tile([P, LOCAL_COLS], bf16)
            for i in range(4):
                w = LOCAL_COLS // 4
                nc.sync.dma_start(out=xt[:, i * w:(i + 1) * w],
                                  in_=rhs_x[:, i * w:(i + 1) * w])
            ar = const.tile([AUGK, LOCAL_COLS], bf16)
            for i in range(2):
                w = LOCAL_COLS // 2
                nc.sync.dma_start(out=ar[:, i * w:(i + 1) * w],
                                  in_=aug_r[:, i * w:(i + 1) * w])
            lhs = const.tile([P, ROWS_PER_CORE], bf16)
            nc.sync.dma_start(out=lhs, in_=lhs_m2)
            augl = const.tile([AUGK, ROWS_PER_CORE], bf16)
            nc.sync.dma_start(out=augl, in_=aug_l)
            ar2 = const.tile([AUGK, LOCAL_COLS], bf16)
            nc.sync.dma_start(out=ar2, in_=aug_r)

            zbias = const.tile([P, 1], f32)
            nc.vector.memset(zbias, 0.0)
            negp = const.tile([P, NPART], f32)

            def consume(t, ps):
                """neg fused reduce of PSUM region ps into column t."""
                fd = ps.shape[-1]
                if t in NEG_ON_ACT:
                    sa = scr_a.tile([P, 2048], f32, tag="sa")
                    nc.scalar.activation(sa[:, :fd], ps, relu, bias=zbias,
                                         scale=-1.0, accum_out=negp[:, t:t + 1])
                else:
                    sv = scr_v.tile([P, 2048], f32, tag="sv")
                    nc.vector.tensor_scalar(sv[:, :fd], ps, 0.0, None, alu_min,
                                            op1=alu_add,
                                            accum_out=negp[:, t:t + 1])

            def mm_group(ps, jj, col0, widths):
                for q, wdt in enumerate(widths):
                    c = col0 + q * 512
                    sl = ps[:, q * 512:q * 512 + wdt]
                    nc.tensor.matmul(sl, lhs[:, jj * P:(jj + 1) * P],
                                     xt[:, c:c + wdt], start=True, stop=False)
                    nc.tensor.matmul(sl, augl[:AUGK, jj * P:(jj + 1) * P],
                                     ar2[:AUGK, c:c + wdt],
                                     start=False, stop=True)

            for jj in range(RB):
                b = jj * P
                ps = psum.tile([P, 2048], f32, tag="ps")
                mm_group(ps, jj, b + 128, (512, 512, 512, 512))
                consume(2 * jj, ps)
                ps = psum.tile([P, 1920], f32, tag="ps")
                mm_group(ps, jj, b + 2176, (512, 512, 512, 384))
                consume(2 * jj + 1, ps)

            # small tiles: (jj, d0) and (jj, d32) blocks, 4 per PSUM tile
            for s in range(RB // 2):
                ps = psum.tile([P, 512], f32, tag="ps")
                for q in range(4):
                    jj = 2 * s + q // 2
                    col0 = jj * P + (0 if q % 2 == 0 else 4096)
                    sl = ps[:, q * P:(q + 1) * P]
                    nc.tensor.matmul(sl, lhs[:, jj * P:(jj + 1) * P],
                                     xt[:, col0:col0 + P],
                                     start=True, stop=False)
                    nc.tensor.matmul(sl, augl[:AUGK, jj * P:(jj + 1) * P],
                                     ar2[:AUGK, col0:col0 + P],
                                     start=False, stop=True)
                consume(2 * RB + s, ps)

            nc.sync.dma_start(out=neg_out, in_=negp)

    nc.compile()
    return nc


def _prep_inputs(x: np.ndarray, y: np.ndarray):
    """Host-side shard prep. O(N*D) only."""
    import ml_dtypes
    bf = ml_dtypes.bfloat16

    x = np.ascontiguousarray(np.asarray(x, dtype=np.float32))
    y = np.asarray(y).astype(np.int64)
    assert x.shape == (N, D) and y.shape == (N,)

    # Round x to bf16 first, then derive sq from the *rounded* x so the
    # device-side distance geometry is self-consistent (diag lands at ~0).
    xb = x.astype(bf)
    xf = xb.astype(np.float32)
    sq = (xf * xf).sum(axis=1, dtype=np.float32)          # [N]
    oh = np.zeros((C, N), dtype=np.float32)
    oh[y, np.arange(N)] = 1.0

    xT = np.ascontiguousarray(xb.T)                       # [128, N] bf16

    def hi_lo(v):
        hi = v.astype(bf).astype(np.float32)
        lo = v - hi
        return hi, lo

    sq_hi, sq_lo = hi_lo(sq)
    sm1_hi, sm1_lo = hi_lo(sq - 1.0)

    # u += BIG*eq + sq_j + (sq_i - 1): rows 43/44 carry sq_j (hi+lo, lhs=1),
    # rows 45/46 carry sq_i - 1 (hi+lo, rhs=1).
    aug_r = np.empty((AUGK, N), dtype=np.float32)
    aug_r[:C] = BIG * oh
    aug_r[C] = sq_hi
    aug_r[C + 1] = sq_lo
    aug_r[C + 2] = 1.0
    aug_r[C + 3] = 1.0
    aug_r = aug_r.astype(bf)

    aug_l_full = np.empty((AUGK, N), dtype=np.float32)
    aug_l_full[:C] = oh
    aug_l_full[C] = 1.0
    aug_l_full[C + 1] = 1.0
    aug_l_full[C + 2] = sm1_hi
    aug_l_full[C + 3] = sm1_lo
    aug_l_full = aug_l_full.astype(bf)

    in_maps = []
    for k in range(NCORES):
        r0 = k * ROWS_PER_CORE
        idx = (r0 + np.arange(LOCAL_COLS)) % N
        rows = slice(r0, r0 + ROWS_PER_CORE)
        in_maps.append({
            "rhs_x": np.ascontiguousarray(xT[:, idx]),
            "aug_r": np.ascontiguousarray(aug_r[:, idx]),
            "lhs_m2": np.ascontiguousarray(-2.0 * xT[:, rows].astype(np.float32)).astype(bf),
            "aug_l": np.ascontiguousarray(aug_l_full[:, rows]),
        })

    cnt = np.bincount(y, minlength=C).astype(np.float64)
    sum_sq_cnt = float((cnt * cnt).sum())
    pos_cnt = sum_sq_cnt - N
    neg_cnt = float(N) * N - sum_sq_cnt

    # pos term via the O(N*D) identity (exact in f64 on the bf16-rounded x):
    #   sum_{eq pairs} (sq_i + sq_j - 2 x_i.x_j)
    #     = 2 sum_i sq_i*cnt[y_i] - 2 sum_c ||sum_{i in c} x_i||^2
    # (diagonal contributes exactly 0, matching the reference's eq - I mask.)
    x64 = xf.astype(np.float64)
    sq64 = (x64 * x64).sum(axis=1)
    S = np.zeros((C, D), dtype=np.float64)
    np.add.at(S, y, x64)
    pos_sum = 2.0 * float((sq64 * cnt[y]).sum()) - 2.0 * float((S * S).sum())
    return in_maps, pos_cnt, neg_cnt, pos_sum


def _reduce_outputs(results):
    w = np.asarray(UNIT_W, dtype=np.float64)
    neg_sign = np.where(
        np.isin(np.arange(NPART), list(NEG_ON_ACT)), 1.0, -1.0)
    neg_sum = 0.0
    for r in results:
        neg_sum += float((r["neg_out"].astype(np.float64).sum(axis=0)
                          * w * neg_sign).sum())
    return neg_sum


def kernel(x: np.ndarray, y: np.ndarray) -> np.ndarray:
    in_maps, pos_cnt, neg_cnt, pos_sum = _prep_inputs(x, y)

    if "nc" not in _cache:
        _cache["nc"] = _build_bass()
    nc = _cache["nc"]

    res = run_bass_kernel_spmd(nc, in_maps, core_ids=list(range(NCORES)),
                               trace=TRACE)
    _cache["last_results"] = res

    neg_sum = _reduce_outputs(res.results)
    loss = (pos_sum / pos_cnt + neg_sum / neg_cnt) / 2.0
    return np.float32(loss)



# revision 4
# speedup vs baseline: 2.2971x; 2.2971x over previous
"""Contrastive loss on Trainium2 (8 NeuronCores, SPMD, Bass/Tile).

Math
----
reference:
    norms[i,j] = ||x_i||^2 + ||x_j||^2 - 2 x_i.x_j
    pos = sum((eq - I) * norms) / cnt_pos          eq[i,j] = [y_i == y_j]
    neg = sum((1 - eq) * relu(1 - norms)) / cnt_neg
    loss = (pos + neg) / 2

pos is computed exactly on the host via the class-sum identity (O(N*D)).
The device computes only the masked negative sum.

Device trick (fp8 DoubleRow, ONE matmul per output tile):
    u[i,j] = norms[i,j] - 1 + BIG * eq[i,j]          (BIG = 64 >= 1 + fp8 noise)

A single K=256 fp8 DoubleRow matmul packs both halves:
  - K-half 0 (128 rows): lhsT = -2 x_i^T, rhs = x_j^T          -> -2 G
  - K-half 1 (47 rows + zero pad): lhsT = [onehot; 1; sm1 hi/lo],
    rhs = [BIG*onehot; sq hi/lo; 1]                            -> BIG*eq + sq_j + (sq_i - 1)

Both masked sums come out of u with ONE fused instruction each:
    neg (ACT): sum relu(-u) = +sum_{eq=0} relu(1-norms)   (accum_out)
    neg (DVE): sum min(u,0) = -sum_{eq=0} relu(1-norms)   (accum_out)
eq pairs (incl. diagonal) land at u ~ d2-1+64 > 0 -> contribute 0.
fp8 margins (verified on data): min off-diag d2 ~ 121 >> 1, max value 205 < 240.

Work halving (symmetry): with 128-row blocks r and 128-col blocks c (64 of
each), let d = (c - r) mod 64. Summing blocks d=0 (weight 1), d=1..31
(weight 2), d=32 (weight 1) covers every ordered pair exactly once. Each
row-block processes a contiguous circular span of 33*128 = 4224 columns.

Sharding: core k owns global rows [1024k, 1024(k+1)). Its 8 row-blocks need
the circular column window [1024k, 1024k + 5120) — the host ships that
window per-core ("rolled" columns), so the device program is identical on
every core (pure SPMD). Per-core outputs are per-partition partial sums;
the host applies unit weights / counts and reduces (O(N) work).
"""

import numpy as np
from contextlib import ExitStack

import concourse.bass as bass
import concourse.bacc as bacc
import concourse.tile as tile
from concourse import mybir
from concourse.bass_utils import run_bass_kernel_spmd

N, D, C = 8192, 128, 43
MARGIN = 1.0
BIG = 64.0
P = 128
NCORES = 8
ROWS_PER_CORE = N // NCORES           # 1024
RB = ROWS_PER_CORE // P               # 8 row-blocks per core
LOCAL_COLS = ROWS_PER_CORE + 32 * P   # 5120: own rows + 32 blocks ahead
AUGK = C + 4                          # 47 aug rows (onehot + sq hi/lo + ones)
WARMUP = 16                           # PE warm-up matmuls during DMA wait

# Per row-block jj (local col base b = 128*jj) the 4224-col span splits as:
#   M0: [b, b+1024)        d0 block cols [0:128) weight 1, rest weight 2
#   M1: [b+1024, b+2048)   weight 2
#   M2: [b+2048, b+3072)   weight 2
#   M3: [b+3072, b+4096)   weight 2
#   S : [b+4096, b+4224)   d32 block, weight 1
# Units per jj: 6 (M0 split into d0 + rest), NPART = 48.
UNITS_PER_JJ = 6
NPART = UNITS_PER_JJ * RB


def _unit_info():
    """Per unit: (weight, sign). sign=+1 for ACT (relu(-u)), -1 for DVE."""
    w = np.zeros(NPART)
    s = np.zeros(NPART)
    for jj in range(RB):
        u = UNITS_PER_JJ * jj
        w[u + 0], s[u + 0] = 1.0, +1.0   # d0      (ACT)
        w[u + 1], s[u + 1] = 2.0, +1.0   # M0 rest (ACT)
        w[u + 2], s[u + 2] = 2.0, -1.0   # M1      (DVE)
        w[u + 3], s[u + 3] = 2.0, +1.0   # M2      (ACT)
        w[u + 4], s[u + 4] = 2.0, -1.0   # M3      (DVE)
        w[u + 5] = 1.0                   # d32     (alternate)
        s[u + 5] = +1.0 if jj % 2 == 0 else -1.0
    return w, s


UNIT_W, UNIT_SIGN = _unit_info()

_cache = {}
TRACE = False


def _build_bass():
    f32 = mybir.dt.float32
    fp8 = mybir.dt.float8e4
    nc = bacc.Bacc("TRN2", target_bir_lowering=False, debug=False)

    rx_d = nc.dram_tensor("rx", [P, 2, LOCAL_COLS], fp8, kind="ExternalInput").ap()
    wt_d = nc.dram_tensor("wt", [P, 2, ROWS_PER_CORE], fp8, kind="ExternalInput").ap()
    neg_out = nc.dram_tensor("neg_out", [P, NPART], f32, kind="ExternalOutput").ap()

    relu = mybir.ActivationFunctionType.Relu
    alu_min = mybir.AluOpType.min
    alu_add = mybir.AluOpType.add
    DR = mybir.MatmulPerfMode.DoubleRow

    with tile.TileContext(nc) as tc:
        with ExitStack() as ctx:
            const = ctx.enter_context(tc.tile_pool(name="const", bufs=1))
            psum = ctx.enter_context(tc.tile_pool(name="psum", bufs=3, space="PSUM"))
            psum_s = ctx.enter_context(tc.tile_pool(name="psum_s", bufs=2, space="PSUM"))
            scr_a = ctx.enter_context(tc.tile_pool(name="scr_a", bufs=2))
            scr_v = ctx.enter_context(tc.tile_pool(name="scr_v", bufs=2))

            # ---- constants / weights ----
            wu = const.tile([P, 32], fp8)          # warm-up weights
            nc.vector.memset(wu, 0.0)
            zbias = const.tile([P, 1], f32)
            nc.vector.memset(zbias, 0.0)
            negp = const.tile([P, NPART], f32)

            rxt = const.tile([P, 2, LOCAL_COLS], fp8)
            wt = const.tile([P, 2, ROWS_PER_CORE], fp8)

            # rx aug-half rows [AUGK:] are multiplied by zero weights, but
            # garbage bits could be NaN (0*NaN = NaN) -> zero them once.
            # (gpsimd handles <=32 partitions per op at nonzero base; rows
            # 32:47 are re-written by the aug DMA afterwards.)
            for pb in range(32, P, 32):
                nc.gpsimd.memset(rxt[pb:pb + 32, 1:2, :], 0.0)

            # ---- input DMAs, split across both HWDGE queues, in need order.
            # x half on sync, aug half + weights on scalar.
            CH = [(0, 2176), (2176, 4224), (4224, LOCAL_COLS)]
            nc.scalar.dma_start(out=wt, in_=wt_d)
            for c0, c1 in CH:
                nc.sync.dma_start(out=rxt[:, 0:1, c0:c1], in_=rx_d[:, 0:1, c0:c1])
                nc.scalar.dma_start(out=rxt[:AUGK, 1:2, c0:c1],
                                    in_=rx_d[:AUGK, 1:2, c0:c1])

            # ---- PE warm-up during DMA wait (HAM un-throttle attempt) ----
            wps = psum_s.tile([P, 128], f32, tag="ps_s")
            for _ in range(WARMUP):
                nc.tensor.matmul(wps[:32, 0:32], wu, wu[:, 0:32],
                                 start=True, stop=True)

            def consume(t, ps, on_act):
                fd = ps.shape[-1]
                if on_act:
                    sa = scr_a.tile([P, 1024], f32, tag="sa")
                    nc.scalar.activation(sa[:, :fd], ps, relu, bias=zbias,
                                         scale=-1.0, accum_out=negp[:, t:t + 1])
                else:
                    sv = scr_v.tile([P, 1024], f32, tag="sv")
                    nc.vector.tensor_scalar(sv[:, :fd], ps, 0.0, None, alu_min,
                                            op1=alu_add,
                                            accum_out=negp[:, t:t + 1])

            # ---- main loop: per row-block, 9 same-weight DoubleRow matmuls
            for jj in range(RB):
                b = jj * P
                u = UNITS_PER_JJ * jj
                wsl = wt[:, :, jj * P:(jj + 1) * P]

                m0 = psum.tile([P, 1024], f32, tag="ps")
                for q in range(2):
                    c = b + q * 512
                    nc.tensor.matmul(m0[:, q * 512:(q + 1) * 512], wsl,
                                     rxt[:, :, c:c + 512],
                                     start=True, stop=True, perf_mode=DR)
                consume(u + 0, m0[:, 0:P], True)
                consume(u + 1, m0[:, P:1024], True)

                m1 = psum.tile([P, 1024], f32, tag="ps")
                for q in range(2):
                    c = b + 1024 + q * 512
                    nc.tensor.matmul(m1[:, q * 512:(q + 1) * 512], wsl,
                                     rxt[:, :, c:c + 512],
                                     start=True, stop=True, perf_mode=DR)
                consume(u + 2, m1, False)

                m2 = psum.tile([P, 1024], f32, tag="ps")
                for q in range(2):
                    c = b + 2048 + q * 512
                    nc.tensor.matmul(m2[:, q * 512:(q + 1) * 512], wsl,
                                     rxt[:, :, c:c + 512],
                                     start=True, stop=True, perf_mode=DR)
                consume(u + 3, m2, True)

                m3 = psum.tile([P, 1024], f32, tag="ps")
                for q in range(2):
                    c = b + 3072 + q * 512
                    nc.tensor.matmul(m3[:, q * 512:(q + 1) * 512], wsl,
                                     rxt[:, :, c:c + 512],
                                     start=True, stop=True, perf_mode=DR)
                consume(u + 4, m3, False)

                ms = psum_s.tile([P, 128], f32, tag="ps_s")
                c = b + 4096
                nc.tensor.matmul(ms, wsl, rxt[:, :, c:c + P],
                                 start=True, stop=True, perf_mode=DR)
                consume(u + 5, ms, jj % 2 == 0)

            nc.sync.dma_start(out=neg_out, in_=negp)

    nc.compile()
    return nc


def _prep_inputs(x: np.ndarray, y: np.ndarray):
    """Host-side shard prep. O(N*D) only."""
    import ml_dtypes
    f8 = ml_dtypes.float8_e4m3fn

    x = np.ascontiguousarray(np.asarray(x, dtype=np.float32))
    y = np.asarray(y).astype(np.int64)
    assert x.shape == (N, D) and y.shape == (N,)

    # fp8-round x; derive sq from the ROUNDED x so device distance geometry
    # is self-consistent (diag lands at ~0, covered by +BIG anyway).
    x8 = x.astype(f8)
    xf = x8.astype(np.float32)
    sq = (xf * xf).sum(axis=1, dtype=np.float32)          # [N]
    oh = np.zeros((C, N), dtype=np.float32)
    oh[y, np.arange(N)] = 1.0

    xT8 = np.ascontiguousarray(x8.T)                      # [128, N] fp8

    def hi_lo(v):
        hi = v.astype(f8).astype(np.float32)
        lo = v - hi
        return hi, lo

    sq_hi, sq_lo = hi_lo(sq)
    sm1_hi, sm1_lo = hi_lo(sq - 1.0)

    # rhs aug rows: BIG*onehot ; sq_j hi/lo (lhs=1) ; ones (lhs=sm1 hi/lo)
    aug_r = np.empty((AUGK, N), dtype=np.float32)
    aug_r[:C] = BIG * oh
    aug_r[C] = sq_hi
    aug_r[C + 1] = sq_lo
    aug_r[C + 2] = 1.0
    aug_r[C + 3] = 1.0
    aug_r = aug_r.astype(f8)

    aug_l = np.empty((AUGK, N), dtype=np.float32)
    aug_l[:C] = oh
    aug_l[C] = 1.0
    aug_l[C + 1] = 1.0
    aug_l[C + 2] = sm1_hi
    aug_l[C + 3] = sm1_lo
    aug_l = aug_l.astype(f8)

    # weights: [128, 2, 1024] per core; x half = -2 x^T (exact in fp8),
    # aug half = aug_l zero-padded to 128 rows.
    m2xT = (-2.0 * xf.T).astype(f8)                       # [128, N], exact

    in_maps = []
    for k in range(NCORES):
        r0 = k * ROWS_PER_CORE
        idx = (r0 + np.arange(LOCAL_COLS)) % N
        rows = slice(r0, r0 + ROWS_PER_CORE)

        rx = np.zeros((P, 2, LOCAL_COLS), dtype=f8)
        rx[:, 0, :] = xT8[:, idx]
        rx[:AUGK, 1, :] = aug_r[:, idx]

        wt = np.zeros((P, 2, ROWS_PER_CORE), dtype=f8)
        wt[:, 0, :] = m2xT[:, rows]
        wt[:AUGK, 1, :] = aug_l[:, rows]

        in_maps.append({"rx": rx, "wt": wt})

    cnt = np.bincount(y, minlength=C).astype(np.float64)
    sum_sq_cnt = float((cnt * cnt).sum())
    pos_cnt = sum_sq_cnt - N
    neg_cnt = float(N) * N - sum_sq_cnt

    # pos term via the O(N*D) identity, f64 on the ORIGINAL f32 x:
    #   sum_{eq pairs} (sq_i + sq_j - 2 x_i.x_j)
    #     = 2 sum_i sq_i*cnt[y_i] - 2 sum_c ||sum_{i in c} x_i||^2
    x64 = x.astype(np.float64)
    sq64 = (x64 * x64).sum(axis=1)
    S = np.zeros((C, D), dtype=np.float64)
    np.add.at(S, y, x64)
    pos_sum = 2.0 * float((sq64 * cnt[y]).sum()) - 2.0 * float((S * S).sum())
    return in_maps, pos_cnt, neg_cnt, pos_sum


def _reduce_outputs(results):
    neg_sum = 0.0
    for r in results:
        neg_sum += float((r["neg_out"].astype(np.float64).sum(axis=0)
                          * UNIT_W * UNIT_SIGN).sum())
    return neg_sum


def kernel(x: np.ndarray, y: np.ndarray) -> np.ndarray:
    in_maps, pos_cnt, neg_cnt, pos_sum = _prep_inputs(x, y)

    if "nc" not in _cache:
        _cache["nc"] = _build_bass()
    nc = _cache["nc"]

    res = run_bass_kernel_spmd(nc, in_maps, core_ids=list(range(NCORES)),
                               trace=TRACE)
    _cache["last_results"] = res

    neg_sum = _reduce_outputs(res.results)
    loss = (pos_sum / pos_cnt + neg_sum / neg_cnt) / 2.0
    return np.float32(loss)


# revision 7
# speedup vs baseline: 2.9433x; 1.2813x over previous
"""Contrastive loss on Trainium2 (8 NeuronCores, SPMD, Bass/Tile).

Math
----
reference:
    norms[i,j] = ||x_i||^2 + ||x_j||^2 - 2 x_i.x_j
    pos = sum((eq - I) * norms) / cnt_pos          eq[i,j] = [y_i == y_j]
    neg = sum((1 - eq) * relu(1 - norms)) / cnt_neg
    loss = (pos + neg) / 2

pos is computed exactly on the host via the class-sum identity (O(N*D)).
The device computes only the masked negative sum.

Device trick (fp8 DoubleRow, ONE matmul per output tile):
    u[i,j] = norms[i,j] - 1 + BIG * eq[i,j]          (BIG = 64 >= 1 + fp8 noise)

A single K=256 fp8 DoubleRow matmul packs both halves:
  - K-half 0 (128 rows): lhsT = -2 x_i^T, rhs = x_j^T          -> -2 G
  - K-half 1 (47 rows + zero pad): lhsT = [onehot; 1; sm1 hi/lo],
    rhs = [BIG*onehot; sq hi/lo; 1]                            -> BIG*eq + sq_j + (sq_i - 1)

Masked sums come out of u with ONE fused instruction per tile:
    ACT:     sum relu(-u)  = +sum_{eq=0} relu(1-norms)   (accum_out)
    DVE/GP:  sum min(u,0)  = -sum_{eq=0} relu(1-norms)   (accum_out)
eq pairs (incl. diagonal) land at u ~ d2-1+64 > 0 -> contribute 0.
fp8 margins (verified on data): min off-diag d2 ~ 121 >> 1, max value 205 < 240.

Work halving (symmetry): with 128-row blocks r and 128-col blocks c (64 of
each), let d = (c - r) mod 64. Summing blocks d=0 (weight 1), d=1..31
(weight 2), d=32 (weight 1) covers every ordered pair exactly once. Each
row-block processes a contiguous circular span of 33*128 = 4224 columns.

Sharding: core k owns global rows [1024k, 1024(k+1)). Its 8 row-blocks need
the circular column window [1024k, 1024k + 5120) — the host ships that
window per-core ("rolled" columns), so the device program is identical on
every core (pure SPMD). Per-core outputs are per-partition partial sums;
the host applies unit weights / counts and reduces (O(N) work).
"""

import numpy as np
from contextlib import ExitStack

import concourse.bass as bass
import concourse.bacc as bacc
import concourse.tile as tile
from concourse import mybir
import concourse.bass_utils as _bu
from concourse.bass_utils import run_bass_kernel_spmd

# walrus disables the LDWEIGHTS dedup pass by default; our inner loop issues
# 9 same-weight matmuls per row-block, so redundant LDW streams cost ~12us.
LDW_OPT = False   # walrus: "InstLdweights is not compatible with LDW optimization"
if LDW_OPT and not getattr(_bu, "_ldw_patch", False):
    _orig_run_command = _bu.run_command

    def _run_command_ldw(cmd, *a, **kw):
        cmd = ["--enable-ldw-opt=true" if c == "--enable-ldw-opt=false" else c
               for c in cmd]
        return _orig_run_command(cmd, *a, **kw)

    _bu.run_command = _run_command_ldw
    _bu._ldw_patch = True

N, D, C = 8192, 128, 43
MARGIN = 1.0
BIG = 64.0
P = 128
NCORES = 8
ROWS_PER_CORE = N // NCORES           # 1024
RB = ROWS_PER_CORE // P               # 8 row-blocks per core
LOCAL_COLS = ROWS_PER_CORE + 32 * P   # 5120: own rows + 32 blocks ahead
AUGK = C + 4                          # 47 aug rows (onehot + sq hi/lo + ones)
WARMUP = 16                           # PE warm-up matmuls during DMA wait
GP_CONSUME = False                    # gpsimd cannot read PSUM on TRN2

# Per row-block jj (local col base b = 128*jj) the 4224-col span splits as:
#   S : [b, b+128) + [b+4096, b+4224)   d0 + d32 blocks, weight 1
#   M0: [b+128, b+1152)    weight 2     (ACT)
#   M1: [b+1152, b+2176)   weight 2     (DVE)
#   M2: [b+2176, b+3200)   weight 2     (ACT)
#   M3: [b+3200, b+4096)   weight 2     (DVE, 896 cols)
UNITS_PER_JJ = 5
NPART = UNITS_PER_JJ * RB


def _unit_info():
    """Per unit: (weight, sign). sign=+1 for ACT relu(-u), -1 for min(u,0)."""
    w = np.zeros(NPART)
    s = np.zeros(NPART)
    for jj in range(RB):
        u = UNITS_PER_JJ * jj
        w[u + 0], s[u + 0] = 2.0, +1.0   # M0 (ACT)
        w[u + 1], s[u + 1] = 2.0, -1.0   # M1 (DVE)
        w[u + 2], s[u + 2] = 2.0, +1.0   # M2 (ACT)
        w[u + 3], s[u + 3] = 2.0, -1.0   # M3 (DVE)
        w[u + 4] = 1.0                   # S d0+d32
        s[u + 4] = -1.0 if GP_CONSUME else (+1.0 if jj % 2 == 0 else -1.0)
    return w, s


UNIT_W, UNIT_SIGN = _unit_info()

_cache = {}
TRACE = False


def _build_bass():
    f32 = mybir.dt.float32
    fp8 = mybir.dt.float8e4
    nc = bacc.Bacc("TRN2", target_bir_lowering=False, debug=False)

    rx_d = nc.dram_tensor("rx", [P, 2, LOCAL_COLS], fp8, kind="ExternalInput").ap()
    wt_d = nc.dram_tensor("wt", [P, 2, ROWS_PER_CORE], fp8, kind="ExternalInput").ap()
    neg_out = nc.dram_tensor("neg_out", [P, NPART], f32, kind="ExternalOutput").ap()

    relu = mybir.ActivationFunctionType.Relu
    alu_min = mybir.AluOpType.min
    alu_add = mybir.AluOpType.add
    DR = mybir.MatmulPerfMode.DoubleRow

    with tile.TileContext(nc) as tc:
        with ExitStack() as ctx:
            const = ctx.enter_context(tc.tile_pool(name="const", bufs=1))
            psum = ctx.enter_context(tc.tile_pool(name="psum", bufs=3, space="PSUM"))
            psum_s = ctx.enter_context(tc.tile_pool(name="psum_s", bufs=2, space="PSUM"))
            scr_a = ctx.enter_context(tc.tile_pool(name="scr_a", bufs=2))
            scr_v = ctx.enter_context(tc.tile_pool(name="scr_v", bufs=2))
            scr_g = ctx.enter_context(tc.tile_pool(name="scr_g", bufs=2))

            # ---- constants / weights ----
            wu = const.tile([P, 32], fp8)          # warm-up weights
            nc.vector.memset(wu, 0.0)
            zbias = const.tile([P, 1], f32)
            nc.vector.memset(zbias, 0.0)
            negp = const.tile([P, NPART], f32)

            rxt = const.tile([P, 2, LOCAL_COLS], fp8)
            wt = const.tile([P, 2, ROWS_PER_CORE], fp8)

            # ---- input DMAs, split across both HWDGE queues, in need order.
            # aug half ships all 128 rows (rows 47.. are zeros baked in DRAM:
            # they meet zero weights, but garbage NaN would poison 0*NaN).
            CH = [(0, 2176), (2176, 4224), (4224, LOCAL_COLS)]
            nc.scalar.dma_start(out=wt, in_=wt_d)
            for c0, c1 in CH:
                nc.sync.dma_start(out=rxt[:, 0:1, c0:c1], in_=rx_d[:, 0:1, c0:c1])
                nc.scalar.dma_start(out=rxt[:, 1:2, c0:c1], in_=rx_d[:, 1:2, c0:c1])

            # ---- PE warm-up during DMA wait (HAM un-throttle) ----
            wps = psum_s.tile([P, 256], f32, tag="ps_s")
            for _ in range(WARMUP):
                nc.tensor.matmul(wps[:32, 0:32], wu, wu[:, 0:32],
                                 start=True, stop=True)

            def consume(t, ps, eng):
                fd = ps.shape[-1]
                if eng == "a":
                    sa = scr_a.tile([P, 1024], f32, tag="sa")
                    nc.scalar.activation(sa[:, :fd], ps, relu, bias=zbias,
                                         scale=-1.0, accum_out=negp[:, t:t + 1])
                else:
                    pool, e = (scr_v, nc.vector) if eng == "v" else (scr_g, nc.gpsimd)
                    sv = pool.tile([P, 1024], f32, tag="s" + eng)
                    e.tensor_scalar(sv[:, :fd], ps, 0.0, None, alu_min,
                                    op1=alu_add, accum_out=negp[:, t:t + 1])

            # ---- main loop: per row-block, 9 same-weight DoubleRow matmuls
            for jj in range(RB):
                b = jj * P
                u = UNITS_PER_JJ * jj
                wsl = wt[:, :, jj * P:(jj + 1) * P]

                # S first: d0 + d32 (weight-1 blocks) in one [P, 256] tile
                ms = psum_s.tile([P, 256], f32, tag="ps_s")
                nc.tensor.matmul(ms[:, 0:P], wsl, rxt[:, :, b:b + P],
                                 start=True, stop=True, perf_mode=DR)
                nc.tensor.matmul(ms[:, P:256], wsl, rxt[:, :, b + 4096:b + 4224],
                                 start=True, stop=True, perf_mode=DR)
                if GP_CONSUME:
                    consume(u + 4, ms, "g")
                else:
                    consume(u + 4, ms, "a" if jj % 2 == 0 else "v")

                for m in range(4):
                    c0 = b + 128 + m * 1024
                    fd = 1024 if m < 3 else 896
                    mt = psum.tile([P, 1024], f32, tag="ps")
                    nc.tensor.matmul(mt[:, 0:512], wsl, rxt[:, :, c0:c0 + 512],
                                     start=True, stop=True, perf_mode=DR)
                    nc.tensor.matmul(mt[:, 512:fd], wsl,
                                     rxt[:, :, c0 + 512:c0 + fd],
                                     start=True, stop=True, perf_mode=DR)
                    consume(u + m, mt[:, :fd], "a" if m % 2 == 0 else "v")

            nc.sync.dma_start(out=neg_out, in_=negp)

    nc.compile()
    return nc


def _prep_inputs(x: np.ndarray, y: np.ndarray):
    """Host-side shard prep. O(N*D) only."""
    import ml_dtypes
    f8 = ml_dtypes.float8_e4m3fn

    x = np.ascontiguousarray(np.asarray(x, dtype=np.float32))
    y = np.asarray(y).astype(np.int64)
    assert x.shape == (N, D) and y.shape == (N,)

    # fp8-round x; derive sq from the ROUNDED x so device distance geometry
    # is self-consistent (diag lands at ~0, covered by +BIG anyway).
    x8 = x.astype(f8)
    xf = x8.astype(np.float32)
    sq = (xf * xf).sum(axis=1, dtype=np.float32)          # [N]
    oh = np.zeros((C, N), dtype=np.float32)
    oh[y, np.arange(N)] = 1.0

    xT8 = np.ascontiguousarray(x8.T)                      # [128, N] fp8

    def hi_lo(v):
        hi = v.astype(f8).astype(np.float32)
        lo = v - hi
        return hi, lo

    sq_hi, sq_lo = hi_lo(sq)
    sm1_hi, sm1_lo = hi_lo(sq - 1.0)

    # rhs aug rows: BIG*onehot ; sq_j hi/lo (lhs=1) ; ones (lhs=sm1 hi/lo)
    aug_r = np.empty((AUGK, N), dtype=np.float32)
    aug_r[:C] = BIG * oh
    aug_r[C] = sq_hi
    aug_r[C + 1] = sq_lo
    aug_r[C + 2] = 1.0
    aug_r[C + 3] = 1.0
    aug_r = aug_r.astype(f8)

    aug_l = np.empty((AUGK, N), dtype=np.float32)
    aug_l[:C] = oh
    aug_l[C] = 1.0
    aug_l[C + 1] = 1.0
    aug_l[C + 2] = sm1_hi
    aug_l[C + 3] = sm1_lo
    aug_l = aug_l.astype(f8)

    # weights: [128, 2, 1024] per core; x half = -2 x^T (exact in fp8),
    # aug half = aug_l zero-padded to 128 rows.
    m2xT = (-2.0 * xf.T).astype(f8)                       # [128, N], exact

    in_maps = []
    for k in range(NCORES):
        r0 = k * ROWS_PER_CORE
        idx = (r0 + np.arange(LOCAL_COLS)) % N
        rows = slice(r0, r0 + ROWS_PER_CORE)

        rx = np.zeros((P, 2, LOCAL_COLS), dtype=f8)
        rx[:, 0, :] = xT8[:, idx]
        rx[:AUGK, 1, :] = aug_r[:, idx]

        wt = np.zeros((P, 2, ROWS_PER_CORE), dtype=f8)
        wt[:, 0, :] = m2xT[:, rows]
        wt[:AUGK, 1, :] = aug_l[:, rows]

        in_maps.append({"rx": rx, "wt": wt})

    cnt = np.bincount(y, minlength=C).astype(np.float64)
    sum_sq_cnt = float((cnt * cnt).sum())
    pos_cnt = sum_sq_cnt - N
    neg_cnt = float(N) * N - sum_sq_cnt

    # pos term via the O(N*D) identity, f64 on the ORIGINAL f32 x:
    #   sum_{eq pairs} (sq_i + sq_j - 2 x_i.x_j)
    #     = 2 sum_i sq_i*cnt[y_i] - 2 sum_c ||sum_{i in c} x_i||^2
    x64 = x.astype(np.float64)
    sq64 = (x64 * x64).sum(axis=1)
    S = np.zeros((C, D), dtype=np.float64)
    np.add.at(S, y, x64)
    pos_sum = 2.0 * float((sq64 * cnt[y]).sum()) - 2.0 * float((S * S).sum())
    return in_maps, pos_cnt, neg_cnt, pos_sum


def _reduce_outputs(results):
    neg_sum = 0.0
    for r in results:
        neg_sum += float((r["neg_out"].astype(np.float64).sum(axis=0)
                          * UNIT_W * UNIT_SIGN).sum())
    return neg_sum


def kernel(x: np.ndarray, y: np.ndarray) -> np.ndarray:
    in_maps, pos_cnt, neg_cnt, pos_sum = _prep_inputs(x, y)

    if "nc" not in _cache:
        _cache["nc"] = _build_bass()
    nc = _cache["nc"]

    res = run_bass_kernel_spmd(nc, in_maps, core_ids=list(range(NCORES)),
                               trace=TRACE)
    _cache["last_results"] = res

    neg_sum = _reduce_outputs(res.results)
    loss = (pos_sum / pos_cnt + neg_sum / neg_cnt) / 2.0
    return np.float32(loss)


# revision 13
# speedup vs baseline: 3.1400x; 1.0668x over previous
"""Contrastive loss on Trainium2 (8 NeuronCores, SPMD, Bass/Tile).

Math
----
reference:
    norms[i,j] = ||x_i||^2 + ||x_j||^2 - 2 x_i.x_j
    pos = sum((eq - I) * norms) / cnt_pos          eq[i,j] = [y_i == y_j]
    neg = sum((1 - eq) * relu(1 - norms)) / cnt_neg
    loss = (pos + neg) / 2

pos is computed exactly on the host via the class-sum identity (O(N*D)).
The device computes only the masked negative sum.

Device trick (fp8 DoubleRow, ONE matmul per output tile):
    u[i,j] = norms[i,j] - 1 + BIG * eq[i,j]          (BIG = 64 >= 1 + fp8 noise)

A single K=256 fp8 DoubleRow matmul packs both halves:
  - K-half 0 (128 rows): lhsT = -2 x_i^T, rhs = x_j^T          -> -2 G
  - K-half 1 (47 rows + zero pad): lhsT = [onehot; 1; sm1 hi/lo],
    rhs = [BIG*onehot; sq hi/lo; 1]                            -> BIG*eq + sq_j + (sq_i - 1)

Masked sums come out of u with ONE fused instruction per tile:
    ACT:     sum relu(-u)  = +sum_{eq=0} relu(1-norms)   (accum_out)
    DVE/GP:  sum min(u,0)  = -sum_{eq=0} relu(1-norms)   (accum_out)
eq pairs (incl. diagonal) land at u ~ d2-1+64 > 0 -> contribute 0.
fp8 margins (verified on data): min off-diag d2 ~ 121 >> 1, max value 205 < 240.

Work halving (symmetry): with 128-row blocks r and 128-col blocks c (64 of
each), let d = (c - r) mod 64. Summing blocks d=0 (weight 1), d=1..31
(weight 2), d=32 (weight 1) covers every ordered pair exactly once. Each
row-block processes a contiguous circular span of 33*128 = 4224 columns.

Sharding: core k owns global rows [1024k, 1024(k+1)). Its 8 row-blocks need
the circular column window [1024k, 1024k + 5120) — the host ships that
window per-core ("rolled" columns), so the device program is identical on
every core (pure SPMD). Per-core outputs are per-partition partial sums;
the host applies unit weights / counts and reduces (O(N) work).
"""

import numpy as np
from contextlib import ExitStack

import concourse.bass as bass
import concourse.bacc as bacc
import concourse.tile as tile
from concourse import mybir
import concourse.bass_utils as _bu
from concourse.bass_utils import run_bass_kernel_spmd

# walrus disables the LDWEIGHTS dedup pass by default; our inner loop issues
# 9 same-weight matmuls per row-block, so redundant LDW streams cost ~12us.
LDW_OPT = False   # walrus: "InstLdweights is not compatible with LDW optimization"
if LDW_OPT and not getattr(_bu, "_ldw_patch", False):
    _orig_run_command = _bu.run_command

    def _run_command_ldw(cmd, *a, **kw):
        cmd = ["--enable-ldw-opt=true" if c == "--enable-ldw-opt=false" else c
               for c in cmd]
        return _orig_run_command(cmd, *a, **kw)

    _bu.run_command = _run_command_ldw
    _bu._ldw_patch = True

N, D, C = 8192, 128, 43
MARGIN = 1.0
BIG = 64.0
P = 128
NCORES = 8
ROWS_PER_CORE = N // NCORES           # 1024
RB = ROWS_PER_CORE // P               # 8 row-blocks per core
LOCAL_COLS = ROWS_PER_CORE + 32 * P   # 5120: own rows + 32 blocks ahead
AUGK = C + 4                          # 47 aug rows (onehot + sq hi/lo + ones)
WARMUP = 26                           # PE warm-up matmuls during DMA wait
GP_CONSUME = False                    # gpsimd cannot read PSUM on TRN2

# Per row-block jj (local col base b = 128*jj) the 4224-col span splits as:
#   S : [b, b+128) + [b+4096, b+4224)   d0 + d32 blocks, weight 1
#   M0: [b+128, b+1152)    weight 2     (ACT)
#   M1: [b+1152, b+2176)   weight 2     (DVE)
#   M2: [b+2176, b+3200)   weight 2     (ACT)
#   M3: [b+3200, b+4096)   weight 2     (DVE, 896 cols)
UNITS_PER_JJ = 5
NPART = UNITS_PER_JJ * RB


def _unit_info():
    """Per unit: (weight, sign). sign=+1 for ACT relu(-u), -1 for min(u,0)."""
    w = np.zeros(NPART)
    s = np.zeros(NPART)
    for jj in range(RB):
        u = UNITS_PER_JJ * jj
        w[u + 0], s[u + 0] = 2.0, +1.0   # M0 (ACT)
        w[u + 1], s[u + 1] = 2.0, -1.0   # M1 (DVE)
        w[u + 2], s[u + 2] = 2.0, +1.0   # M2 (ACT)
        w[u + 3], s[u + 3] = 2.0, -1.0   # M3 (DVE)
        w[u + 4], s[u + 4] = 1.0, -1.0   # S d0+d32 (DVE)
    return w, s


UNIT_W, UNIT_SIGN = _unit_info()

_cache = {}
TRACE = False


def _build_bass():
    f32 = mybir.dt.float32
    fp8 = mybir.dt.float8e4
    nc = bacc.Bacc("TRN2", target_bir_lowering=False, debug=False)

    rx_d = nc.dram_tensor("rx", [P, 2, LOCAL_COLS], fp8, kind="ExternalInput").ap()
    wt_d = nc.dram_tensor("wt", [P, 2, ROWS_PER_CORE], fp8, kind="ExternalInput").ap()
    neg_out = nc.dram_tensor("neg_out", [P, NPART], f32, kind="ExternalOutput").ap()

    relu = mybir.ActivationFunctionType.Relu
    alu_min = mybir.AluOpType.min
    alu_add = mybir.AluOpType.add
    DR = mybir.MatmulPerfMode.DoubleRow

    with tile.TileContext(nc) as tc:
        with ExitStack() as ctx:
            const = ctx.enter_context(tc.tile_pool(name="const", bufs=1))
            psum = ctx.enter_context(tc.tile_pool(name="psum", bufs=3, space="PSUM"))
            psum_s = ctx.enter_context(tc.tile_pool(name="psum_s", bufs=2, space="PSUM"))
            scr_a = ctx.enter_context(tc.tile_pool(name="scr_a", bufs=2))
            scr_v = ctx.enter_context(tc.tile_pool(name="scr_v", bufs=2))
            scr_g = ctx.enter_context(tc.tile_pool(name="scr_g", bufs=2))

            # ---- constants / weights ----
            wu = const.tile([P, 256], fp8)         # warm-up weights/rhs
            nc.vector.memset(wu, 0.0)
            zbias = const.tile([P, 1], f32)
            nc.vector.memset(zbias, 0.0)
            negp = const.tile([P, NPART], f32)

            rxt = const.tile([P, 2, LOCAL_COLS], fp8)
            wt = const.tile([P, 2, ROWS_PER_CORE], fp8)

            # ---- input DMAs, split across both HWDGE queues, in need order.
            # aug half ships all 128 rows (rows 47.. are zeros baked in DRAM:
            # they meet zero weights, but garbage NaN would poison 0*NaN).
            # First chunk is small so jj=0 can start ASAP.
            CH = [(0, 1152), (1152, 2176), (2176, 4224), (4224, LOCAL_COLS)]
            nc.scalar.dma_start(out=wt, in_=wt_d)
            for c0, c1 in CH:
                nc.sync.dma_start(out=rxt[:, 0:1, c0:c1], in_=rx_d[:, 0:1, c0:c1])
                nc.scalar.dma_start(out=rxt[:, 1:2, c0:c1], in_=rx_d[:, 1:2, c0:c1])

            # ---- PE warm-up during DMA wait (HAM un-throttle). FD=256 keeps
            # the PE busy ~3.5us so the HAM SHORT window flips to 2.4 GHz
            # right as the first data lands.
            wps = psum_s.tile([P, 256], f32, tag="ps_s")
            for _ in range(WARMUP):
                nc.tensor.matmul(wps[:32, :], wu[:, 0:32], wu,
                                 start=True, stop=True)

            bf16 = mybir.dt.bfloat16

            def consume(t, ps, eng):
                fd = ps.shape[-1]
                if eng == "a":
                    sa = scr_a.tile([P, 1024], bf16, tag="sa")
                    nc.scalar.activation(sa[:, :fd], ps, relu, bias=zbias,
                                         scale=-1.0, accum_out=negp[:, t:t + 1])
                else:
                    sv = scr_v.tile([P, 1024], bf16, tag="sv")
                    nc.vector.tensor_scalar(sv[:, :fd], ps, 0.0, None, alu_min,
                                            op1=alu_add,
                                            accum_out=negp[:, t:t + 1])

            # ---- main loop: per row-block, 9 same-weight DoubleRow matmuls.
            # S (d0+d32) goes LAST: it needs late columns (b+4096..), and the
            # PE queue is FIFO — issuing it first would gate jj=0 on nearly
            # the whole transfer.
            for jj in range(RB):
                b = jj * P
                u = UNITS_PER_JJ * jj
                wsl = wt[:, :, jj * P:(jj + 1) * P]

                for m in range(4):
                    c0 = b + 128 + m * 1024
                    fd = 1024 if m < 3 else 896
                    mt = psum.tile([P, 1024], f32, tag="ps")
                    nc.tensor.matmul(mt[:, 0:512], wsl, rxt[:, :, c0:c0 + 512],
                                     start=True, stop=True, perf_mode=DR)
                    nc.tensor.matmul(mt[:, 512:fd], wsl,
                                     rxt[:, :, c0 + 512:c0 + fd],
                                     start=True, stop=True, perf_mode=DR)
                    consume(u + m, mt[:, :fd], "a" if m % 2 == 0 else "v")

                ms = psum_s.tile([P, 256], f32, tag="ps_s")
                nc.tensor.matmul(ms[:, 0:P], wsl, rxt[:, :, b:b + P],
                                 start=True, stop=True, perf_mode=DR)
                nc.tensor.matmul(ms[:, P:256], wsl, rxt[:, :, b + 4096:b + 4224],
                                 start=True, stop=True, perf_mode=DR)
                consume(u + 4, ms, "v")

            nc.sync.dma_start(out=neg_out, in_=negp)

    nc.compile()
    return nc


def _prep_inputs(x: np.ndarray, y: np.ndarray):
    """Host-side shard prep. O(N*D) only."""
    import ml_dtypes
    f8 = ml_dtypes.float8_e4m3fn

    x = np.ascontiguousarray(np.asarray(x, dtype=np.float32))
    y = np.asarray(y).astype(np.int64)
    assert x.shape == (N, D) and y.shape == (N,)

    # fp8-round x; derive sq from the ROUNDED x so device distance geometry
    # is self-consistent (diag lands at ~0, covered by +BIG anyway).
    x8 = x.astype(f8)
    xf = x8.astype(np.float32)
    sq = (xf * xf).sum(axis=1, dtype=np.float32)          # [N]
    oh = np.zeros((C, N), dtype=np.float32)
    oh[y, np.arange(N)] = 1.0

    xT8 = np.ascontiguousarray(x8.T)                      # [128, N] fp8

    def hi_lo(v):
        hi = v.astype(f8).astype(np.float32)
        lo = v - hi
        return hi, lo

    sq_hi, sq_lo = hi_lo(sq)
    sm1_hi, sm1_lo = hi_lo(sq - 1.0)

    # rhs aug rows: BIG*onehot ; sq_j hi/lo (lhs=1) ; ones (lhs=sm1 hi/lo)
    aug_r = np.empty((AUGK, N), dtype=np.float32)
    aug_r[:C] = BIG * oh
    aug_r[C] = sq_hi
    aug_r[C + 1] = sq_lo
    aug_r[C + 2] = 1.0
    aug_r[C + 3] = 1.0
    aug_r = aug_r.astype(f8)

    aug_l = np.empty((AUGK, N), dtype=np.float32)
    aug_l[:C] = oh
    aug_l[C] = 1.0
    aug_l[C + 1] = 1.0
    aug_l[C + 2] = sm1_hi
    aug_l[C + 3] = sm1_lo
    aug_l = aug_l.astype(f8)

    # weights: [128, 2, 1024] per core; x half = -2 x^T (exact in fp8),
    # aug half = aug_l zero-padded to 128 rows.
    m2xT = (-2.0 * xf.T).astype(f8)                       # [128, N], exact

    in_maps = []
    for k in range(NCORES):
        r0 = k * ROWS_PER_CORE
        idx = (r0 + np.arange(LOCAL_COLS)) % N
        rows = slice(r0, r0 + ROWS_PER_CORE)

        rx = np.zeros((P, 2, LOCAL_COLS), dtype=f8)
        rx[:, 0, :] = xT8[:, idx]
        rx[:AUGK, 1, :] = aug_r[:, idx]

        wt = np.zeros((P, 2, ROWS_PER_CORE), dtype=f8)
        wt[:, 0, :] = m2xT[:, rows]
        wt[:AUGK, 1, :] = aug_l[:, rows]

        in_maps.append({"rx": rx, "wt": wt})

    cnt = np.bincount(y, minlength=C).astype(np.float64)
    sum_sq_cnt = float((cnt * cnt).sum())
    pos_cnt = sum_sq_cnt - N
    neg_cnt = float(N) * N - sum_sq_cnt

    # pos term via the O(N*D) identity, f64 on the ORIGINAL f32 x:
    #   sum_{eq pairs} (sq_i + sq_j - 2 x_i.x_j)
    #     = 2 sum_i sq_i*cnt[y_i] - 2 sum_c ||sum_{i in c} x_i||^2
    x64 = x.astype(np.float64)
    sq64 = (x64 * x64).sum(axis=1)
    S = np.zeros((C, D), dtype=np.float64)
    np.add.at(S, y, x64)
    pos_sum = 2.0 * float((sq64 * cnt[y]).sum()) - 2.0 * float((S * S).sum())
    return in_maps, pos_cnt, neg_cnt, pos_sum


def _reduce_outputs(results):
    neg_sum = 0.0
    for r in results:
        neg_sum += float((r["neg_out"].astype(np.float64).sum(axis=0)
                          * UNIT_W * UNIT_SIGN).sum())
    return neg_sum


def kernel(x: np.ndarray, y: np.ndarray) -> np.ndarray:
    in_maps, pos_cnt, neg_cnt, pos_sum = _prep_inputs(x, y)

    if "nc" not in _cache:
        _cache["nc"] = _build_bass()
    nc = _cache["nc"]

    res = run_bass_kernel_spmd(nc, in_maps, core_ids=list(range(NCORES)),
                               trace=TRACE)
    _cache["last_results"] = res

    neg_sum = _reduce_outputs(res.results)
    loss = (pos_sum / pos_cnt + neg_sum / neg_cnt) / 2.0
    return np.float32(loss)
